# revision 22
# baseline (speedup 1.0000x reference)
"""Multi-head self-attention TRN2 Bass kernel (v5).

Problem: x[2, 2048, 1024], 16 heads x 64 dim, fp32.
Sharding: 8 cores = 2 batches x 4 head-groups (4 heads each).
Each core computes its batch's partial output (its 4 heads through
QKV -> attention -> output projection rows); host sums the 4 partials
per batch and adds bo.

Engine model (measured on hw):
  - every matmul streams one moving column per PE cycle (216ns per 512
    cols at 2.4GHz) regardless of dtype; fp8e4 DoubleRow packs TWO
    K-chunks per instruction, halving deep-contraction GEMMs only.
  - ACT exp runs ~1010ns per [128,2,512] tile; 128 exp tiles (~130us)
    and the PE stream (QKV 41 + scores 54.6 + AV-DR 27.3 + out_proj
    13.7 = ~139us) are co-critical.
  - all input DMAs ride one ~350GB/s SP queue in program order.

Scheduling:
  - DMA need-order: bv,bq,wk,x0,wq,wv,x1,x2,x3,wo with host-side
    pre-transposed weights (contiguous 4KB/partition descriptors).
  - QKV / out_proj fillers are <=432ns thunks in a deadline-sorted
    queue, drained at most ~1 per score iteration so the PE never puts
    a multi-us blob in front of the scores feeding the ACT exp stream.
  - attn@V: fp8e4 DoubleRow over k-chunk pairs.  V is stored f8 as
    [128, 16, head, 80] with a fused ones-column (softmax sums) at col
    64; cols 65-79 zeroed: the dual-fp8 weight loader reads the whole
    16B-aligned window, so everything it can touch must be finite.
    HAZARD (hw): the compiler splits DR matmuls into Ldweights+Matmult
    with the semaphore wait on the Matmult, and the PE prefetches the
    Ldweights while the previous matmul streams -- a V chunk written by
    DVE too close to its first DR read is read STALE.  v-thunks
    therefore carry earliest/deadline windows keeping every write >=6
    iterations ahead of its read (validated with an SBUF NaN-scrambler).
  - exp writes f8 attnT directly with scale=4 (Wq carries 1/32).
  - normalize: sums row -> fast reciprocal -> rank-1 PE broadcast
    (216ns) -> DVE mul; deferred into the next pair's early iterations.
  - tail: DVE+ACT split evacuations, PE broadcast, out DMAs in halves.
"""

import numpy as np

S = 2048          # sequence length per batch
H = 1024          # hidden
G = 256           # head-group width (4 heads x 64)
HD = 65           # V' columns per head (64 + ones)
VPAD = 80         # padded V stride (16B-aligned for DoubleRow weights)
NHL = 4           # heads per core
N_CORES = 8

_CACHE = {}


def _build():
    if "nc" in _CACHE:
        return _CACHE["nc"]

    import concourse.bass as bass
    import concourse.mybir as mybir
    import concourse.tile as tile
    from concourse import bacc
    from concourse.tile_rust import add_dep_helper

    f32 = mybir.dt.float32
    bf16 = mybir.dt.bfloat16
    f8 = mybir.dt.float8e4
    EXP = mybir.ActivationFunctionType.Exp
    DR = mybir.MatmulPerfMode.DoubleRow

    nc = bacc.Bacc("TRN2", target_bir_lowering=False, debug=False,
                   num_devices=N_CORES)

    xt_in = nc.dram_tensor("xt", [4, 128, 8, 512], bf16, kind="ExternalInput")
    wq_in = nc.dram_tensor("wq", [128, 8, G], bf16, kind="ExternalInput")
    wk_in = nc.dram_tensor("wk", [128, 8, G], bf16, kind="ExternalInput")
    wv_in = nc.dram_tensor("wv", [128, 8, G], bf16, kind="ExternalInput")
    bq_in = nc.dram_tensor("bq", [128, 2, 1], f32, kind="ExternalInput")
    bv_in = nc.dram_tensor("bv", [1, G], f32, kind="ExternalInput")
    wo_in = nc.dram_tensor("wo", [128, 2, H], bf16, kind="ExternalInput")
    out_d = nc.dram_tensor("out", [S, H], bf16, kind="ExternalOutput")

    with tile.TileContext(nc) as tc:
        with tc.tile_pool(name="persist", bufs=1) as persist:
            qT = persist.tile([128, 2, S], bf16)     # [qd, m, s]
            kT = persist.tile([128, 2, S], bf16)
            vp = persist.tile([128, 16, NHL, VPAD], f8)  # [s-part, st, h, c]
            bq_sb = persist.tile([128, 2, 1], f32)
            bv_f = persist.tile([1, G], f32)
            bv_bf = persist.tile([1, G], bf16)
            bv_bc = persist.tile([128, G], f32)
            wo_pr = persist.tile([128, 2, H], bf16)
            ones64 = persist.tile([1, 64], bf16)
            ones128 = persist.tile([1, 128], bf16)
            warm = persist.tile([128, 512], bf16)
            warm_e = persist.tile([1, 8], f32)
            wq_sb = persist.tile([128, 8, G], bf16)
            wk_sb = persist.tile([128, 8, G], bf16)
            wv_sb = persist.tile([128, 8, G], bf16)
            xTc = [persist.tile([128, 8, 512], bf16, name=f"xT_{jc}")
                   for jc in range(4)]

            with (
                tc.tile_pool(name="at_roll", bufs=2) as at_pool,
                tc.tile_pool(name="outP", bufs=4) as op_pool,
                tc.tile_pool(name="tmpo", bufs=1) as tmpo_pool,
                tc.tile_pool(name="sums", bufs=4) as sums_pool,
                tc.tile_pool(name="osb", bufs=2) as osb_pool,
                tc.tile_pool(name="ps_s", bufs=2, space="PSUM") as ps_s_pool,
                tc.tile_pool(name="ps_av", bufs=2, space="PSUM") as ps_av_pool,
                tc.tile_pool(name="ps_op", bufs=1, space="PSUM") as ps_op_pool,
            ):
                # -------- DMAs: two hw queues (SP + ACT), need-order,
                # NO dep chains (each chained transfer would serialize its
                # ~1.8us init; ring FIFO preserves per-queue order anyway)
                # SP queue: warmup-critical (weights + x0)
                nc.sync.dma_start(out=bv_f, in_=bv_in.ap())
                nc.sync.dma_start(out=bq_sb, in_=bq_in.ap())
                nc.sync.dma_start(out=wk_sb, in_=wk_in.ap())
                nc.sync.dma_start(out=xTc[0][:, 0:4, :],
                                  in_=xt_in.ap()[0][:, 0:4, :])
                nc.sync.dma_start(out=xTc[0][:, 4:8, :],
                                  in_=xt_in.ap()[0][:, 4:8, :])
                nc.sync.dma_start(out=wq_sb, in_=wq_in.ap())
                nc.sync.dma_start(out=wv_sb, in_=wv_in.ap())
                # ACT queue: pair-0 streaming (x1-x3) + wo
                nc.scalar.dma_start(out=xTc[1], in_=xt_in.ap()[1])
                nc.scalar.dma_start(out=xTc[2], in_=xt_in.ap()[2])
                nc.scalar.dma_start(out=xTc[3], in_=xt_in.ap()[3])
                nc.scalar.dma_start(out=wo_pr, in_=wo_in.ap())

                # memsets after the DMA kicks so they don't gate the queue
                nc.gpsimd.memset(warm, 0.125)
                # dual-fp8 ldweights reads whole padded windows: all of vp
                # must be finite before any AV matmul
                nc.gpsimd.memset(vp, 0.0)
                nc.gpsimd.memset(vp[:, :, :, 64:65], 1.0)
                nc.gpsimd.memset(ones64, 1.0)
                nc.gpsimd.memset(ones128, 1.0)

                # pre-load the exp activation table off the critical path
                nc.scalar.activation(out=warm_e, in_=warm[0:1, 0:8], func=EXP,
                                     scale=4.0)

                def dummy(n=512):
                    ps_d = ps_op_pool.tile([128, 512], f32, tag="dummy",
                                           bufs=1)
                    nc.tensor.matmul(ps_d[:, 0:n], lhsT=warm[:, 0:128],
                                     rhs=warm[:, 0:n], start=True, stop=True)

                for _ in range(4):
                    dummy()
                # bv broadcast along partitions via rank-1 PE outer product
                nc.vector.tensor_copy(bv_bf, bv_f)
                ps_bv = ps_op_pool.tile([128, G], f32, tag="oproj",
                                        name="ps_bv", bufs=1)
                nc.tensor.matmul(ps_bv, lhsT=ones128, rhs=bv_bf,
                                 start=True, stop=True)
                nc.vector.tensor_copy(bv_bc, ps_bv)
                for _ in range(4):
                    dummy()

                # ---------------- QKV building blocks ----------------
                qk_ring = [0]

                def ring_tag():
                    t = "dummy" if qk_ring[0] == 0 else "oproj"
                    qk_ring[0] ^= 1
                    return t

                def qk_start(w_sb, jc, m):
                    return ps_op_pool.tile([128, 512], f32, tag=ring_tag(),
                                           name=f"psqk_{id(w_sb)}_{jc}_{m}",
                                           bufs=1)

                def qk_mm(ps, w_sb, jc, m, ht0, ht1):
                    for ht in range(ht0, ht1):
                        nc.tensor.matmul(
                            ps,
                            lhsT=w_sb[:, ht, m * 128:(m + 1) * 128],
                            rhs=xTc[jc][:, ht, :],
                            start=(ht == 0), stop=(ht == 7))

                def qk_evac(ps, b_sb, dst, jc, m):
                    sl = slice(jc * 512, (jc + 1) * 512)
                    if b_sb is not None:
                        nc.vector.tensor_scalar_add(dst[:, m, sl], ps,
                                                    b_sb[:, m, :])
                    else:
                        nc.vector.tensor_copy(dst[:, m, sl], ps)

                def v_mm(ps, st16, ht0, ht1):
                    for ht in range(ht0, ht1):
                        nc.tensor.matmul(
                            ps[:, 0:G],
                            lhsT=xTc[st16 // 4][:, ht,
                                                (st16 % 4) * 128:
                                                (st16 % 4 + 1) * 128],
                            rhs=wv_sb[:, ht, :],
                            start=(ht == 0), stop=(ht == 7))

                def v_evac(ps, st16):
                    nc.vector.tensor_add(
                        vp[:, st16, :, 0:64],
                        ps[:, 0:G].rearrange("p (h d) -> p h d", h=NHL),
                        bv_bc.rearrange("p (h d) -> p h d", h=NHL))

                # thunk queue entries: (deadline, earliest, cost_ns, fn)
                def enq_qk(fq, w_sb, b_sb, dst, jc, m, deadline, earliest=0):
                    box = []

                    def t0():
                        box.append(qk_start(w_sb, jc, m))
                        qk_mm(box[0], w_sb, jc, m, 0, 2)
                    fq.append((deadline, earliest, 432, t0))
                    for h0 in (2, 4):
                        fq.append((deadline, earliest, 432,
                                   lambda h0=h0: qk_mm(box[0], w_sb, jc, m,
                                                       h0, h0 + 2)))

                    def t3():
                        qk_mm(box[0], w_sb, jc, m, 6, 8)
                        qk_evac(box[0], b_sb, dst, jc, m)
                    fq.append((deadline, earliest, 432, t3))

                def enq_v(fq, st16, deadline, earliest=0):
                    box = []

                    def t0():
                        box.append(ps_op_pool.tile([128, 512], f32,
                                                   tag=ring_tag(),
                                                   name=f"psv_{st16}",
                                                   bufs=1))
                        v_mm(box[0], st16, 0, 4)
                    fq.append((deadline, earliest, 432, t0))

                    def t1():
                        v_mm(box[0], st16, 4, 8)
                        v_evac(box[0], st16)
                    fq.append((deadline, earliest, 432, t1))

                def enq_oproj(fq, qc, outPs, qt, deadline, earliest):
                    box = []

                    def mk(ncx):
                        def t():
                            if ncx == 0:
                                box.append(osb_pool.tile(
                                    [128, H], bf16, tag="osb",
                                    name=f"osb_{qc}_{qt}"))
                            osb = box[0]
                            ps_op = ps_op_pool.tile(
                                [128, 512], f32, tag=ring_tag(),
                                name=f"pso_{qc}_{qt}_{ncx}", bufs=1)
                            for pr in range(2):
                                nc.tensor.matmul(
                                    ps_op,
                                    lhsT=outPs[pr][:, qt * 128:(qt + 1) * 128],
                                    rhs=wo_pr[:, pr, ncx * 512:(ncx + 1) * 512],
                                    start=(pr == 0), stop=(pr == 1))
                            nc.vector.tensor_copy(
                                osb[:, ncx * 512:(ncx + 1) * 512], ps_op)
                            nc.sync.dma_start(
                                out=out_d.ap()[qc * 512 + qt * 128:
                                               qc * 512 + (qt + 1) * 128,
                                               ncx * 512:(ncx + 1) * 512],
                                in_=osb[:, ncx * 512:(ncx + 1) * 512])
                        return t
                    fq.append((deadline, earliest, 500, mk(0)))
                    fq.append((deadline, earliest, 500, mk(1)))

                # warmup: kT jc0 m0, qT qc0 m0, v st 0-5 (x0/x1-gated; the
                # first streamed v-thunk (st6) is read at iteration 9)
                ps_w = qk_start(wk_sb, 0, 0)
                qk_mm(ps_w, wk_sb, 0, 0, 0, 8)
                qk_evac(ps_w, None, kT, 0, 0)
                ps_w2 = qk_start(wq_sb, 0, 0)
                qk_mm(ps_w2, wq_sb, 0, 0, 0, 8)
                qk_evac(ps_w2, bq_sb, qT, 0, 0)
                for st in range(6):
                    ps_v = ps_op_pool.tile([128, 512], f32, tag=ring_tag(),
                                           name=f"psvw_{st}", bufs=1)
                    v_mm(ps_v, st, 0, 8)
                    v_evac(ps_v, st)

                # ---------------- normalize helpers ----------------
                def norm_evac(ps_av, hh, tag):
                    uout = tmpo_pool.tile([HD, 512], f32, tag="uout",
                                          name=f"uo_{tag}_{hh}", bufs=4)
                    nc.vector.tensor_copy(uout, ps_av)
                    return uout

                def norm_recip(uout, hh, tag):
                    sums = sums_pool.tile([1, 512], f32, tag="sums",
                                          name=f"sm_{tag}_{hh}")
                    nc.vector.tensor_copy(sums, uout[64:65, :])
                    recip = sums_pool.tile([1, 512], f32, tag="recip",
                                           name=f"rc_{tag}_{hh}")
                    nc.vector.reciprocal_approx_fast(out=recip, in_=sums)
                    recip_bf = sums_pool.tile([1, 512], bf16, tag="recipb",
                                              name=f"rcb_{tag}_{hh}")
                    nc.vector.tensor_copy(recip_bf, recip)
                    return recip_bf

                def norm_fin(outP, uout, recip_bf, hh, tag):
                    # rank-1 PE broadcast (216ns) instead of the ~1us gpsimd
                    # partition_broadcast
                    ps_r = ps_op_pool.tile([128, 512], f32, tag=ring_tag(),
                                           name=f"psrb_{tag}_{hh}", bufs=1)
                    nc.tensor.matmul(ps_r[0:64, :], lhsT=ones64,
                                     rhs=recip_bf, start=True, stop=True)
                    nc.vector.tensor_mul(
                        outP[hh * 64:hh * 64 + 64, :], uout[0:64, :],
                        ps_r[0:64, :])

                # ---------------- attention sweep ----------------
                fq = []
                BUDGET = 200  # pops exactly one ~432ns thunk per iteration

                def drain(g, budget):
                    spent = 0
                    while fq:
                        dl, ea, cost, fn = fq[0]
                        if dl <= g:
                            pass              # forced
                        elif spent < budget and ea <= g:
                            pass              # budgeted
                        else:
                            break
                        fq.pop(0)
                        fn()
                        spent += cost

                def enq(items):
                    fq.extend(items)
                    fq.sort(key=lambda e: e[0])

                pending_norm = None
                pairs = []
                for qc in range(4):
                    for mt in ((1, 0) if qc == 3 else (0, 1)):
                        pairs.append((qc, mt))

                outPs0 = None
                for pair_idx, (qc, mt) in enumerate(pairs):
                    g0 = pair_idx * 16
                    qsl = slice(qc * 512, (qc + 1) * 512)
                    tag = f"{qc}_{mt}"

                    tmp = []
                    if pair_idx == 0:
                        # v st6-15: earliest keeps each write >=6 iterations
                        # ahead of its first DR read (at kt = st - st%2 + 3)
                        for st in range(6, 16):
                            rd = st - (st % 2) + 3
                            enq_v(tmp, st, max(rd - 6, 1),
                                  earliest=max(rd - 9, 0))
                        for jc in range(1, 4):
                            enq_qk(tmp, wk_sb, None, kT, jc, 0, 4 * jc - 1)
                        enq_qk(tmp, wk_sb, None, kT, 0, 1, 14, earliest=8)
                    if pair_idx == 1:
                        for jc in range(1, 4):
                            enq_qk(tmp, wk_sb, None, kT, jc, 1,
                                   16 + 4 * jc - 1)
                    if pair_idx + 1 < len(pairs):
                        nqc, nmt = pairs[pair_idx + 1]
                        enq_qk(tmp, wq_sb, bq_sb, qT, nqc, nmt,
                               g0 + 14 if pair_idx == 0 else g0 + 16,
                               earliest=(10 if pair_idx == 0 else g0))
                    enq(tmp)

                    attnT = at_pool.tile([128, 2, 4, 512], f8,
                                         tag="at", name=f"at_{tag}")
                    ps_avs = [ps_av_pool.tile([HD, 512], f32, tag="av",
                                              name=f"av_{tag}_{hh}")
                              for hh in range(2)]

                    def av_dr(t, ps_avs=ps_avs, attnT=attnT, mt=mt):
                        # fp8 DoubleRow: k-chunks (2t, 2t+1) per instruction.
                        # NOTE: the dual-fp8 Ldweights is prefetched before
                        # its semaphore wait fires -- vp writes must stay
                        # well ahead of their first read (see enq_v windows).
                        sl4 = (2 * t) % 4
                        for hh in range(2):
                            nc.tensor.matmul(
                                ps_avs[hh],
                                lhsT=vp[:, 2 * t:2 * t + 2,
                                        2 * mt + hh, 0:HD],
                                rhs=attnT[:, hh, sl4:sl4 + 2, :],
                                start=(t == 0), stop=(t == 7),
                                perf_mode=DR)

                    for kt in range(16):
                        g = g0 + kt
                        # scores+exp FIRST so the ACT stream is never gated
                        # by fillers/normalize emitted this iteration
                        ps_s = ps_s_pool.tile([128, 2, 512], f32, tag="s")
                        for hh in range(2):
                            nc.tensor.matmul(
                                ps_s[:, hh, :],
                                lhsT=kT[hh * 64:hh * 64 + 64, mt,
                                        kt * 128:(kt + 1) * 128],
                                rhs=qT[hh * 64:hh * 64 + 64, mt, qsl],
                                start=True, stop=True)
                        nc.scalar.activation(
                            out=attnT[:, :, kt % 4, :], in_=ps_s,
                            func=EXP, scale=4.0)
                        if kt >= 3 and kt % 2 == 1:
                            av_dr((kt - 3) // 2)
                        # deferred normalize of the previous pair
                        if pending_norm is not None:
                            pP, pavs, puo, prc, ptag, pdone = pending_norm
                            if kt in (0, 1):
                                puo.append(norm_evac(pavs[kt], kt, ptag))
                            elif kt in (2, 3):
                                prc.append(norm_recip(puo[kt - 2],
                                                      kt - 2, ptag))
                            elif kt == 4:
                                norm_fin(pP, puo[0], prc[0], 0, ptag)
                            elif kt == 5:
                                norm_fin(pP, puo[1], prc[1], 1, ptag)
                                pending_norm = None
                                if pdone is not None:
                                    pq, pouts = pdone
                                    tmp = []
                                    for qt in range(4):
                                        enq_oproj(tmp, pq, pouts, qt,
                                                  min(g0 + 18 + 2 * qt, 126),
                                                  earliest=g + 3 + 2 * qt)
                                    enq(tmp)
                        # forced (deadline<=g: needed before scores(g+1)),
                        # then one budgeted thunk
                        drain(g, BUDGET)
                    av_dr(7)

                    outP = op_pool.tile([128, 512], bf16, tag="outP",
                                        name=f"outP_{tag}")
                    if pair_idx % 2 == 1:
                        done = (qc, [outPs0, outP] if mt == 1
                                else [outP, outPs0])
                    else:
                        outPs0 = outP
                        done = None
                    pending_norm = (outP, ps_avs, [], [], tag, done)

                # ---------------- tail ----------------
                pP, pavs, puo, prc, ptag, pdone = pending_norm
                uo0 = tmpo_pool.tile([HD, 512], f32, tag="uout",
                                     name="uo_t0", bufs=4)
                nc.vector.tensor_copy(uo0, pavs[0])
                uo1 = tmpo_pool.tile([HD, 512], f32, tag="uout",
                                     name="uo_t1", bufs=4)
                nc.scalar.copy(uo1, pavs[1])
                rb0 = norm_recip(uo0, 0, "t0")
                rb1 = norm_recip(uo1, 1, "t1")
                ps_b = ps_op_pool.tile([128, 512], f32, tag="dummy",
                                       name="ps_bcast", bufs=1)
                nc.tensor.matmul(ps_b[0:64, :], lhsT=ones64, rhs=rb0,
                                 start=True, stop=True)
                nc.tensor.matmul(ps_b[64:128, :], lhsT=ones64, rhs=rb1,
                                 start=True, stop=True)
                nc.vector.tensor_mul(pP[0:64, :], uo0[0:64, :], ps_b[0:64, :])
                nc.vector.tensor_mul(pP[64:128, :], uo1[0:64, :],
                                     ps_b[64:128, :])

                assert pdone is not None
                pq, pouts = pdone
                drain(10 ** 9, 10 ** 9)
                for qt in range(4):
                    osb = osb_pool.tile([128, H], bf16, tag="osb",
                                        name=f"osbt_{qt}")
                    for ncx in range(2):
                        ps_op = ps_s_pool.tile(
                            [128, 2, 512], f32, tag="s",
                            name=f"psot_{qt}_{ncx}")[:, 0, :]
                        for pr in range(2):
                            nc.tensor.matmul(
                                ps_op,
                                lhsT=pouts[pr][:, qt * 128:(qt + 1) * 128],
                                rhs=wo_pr[:, pr, ncx * 512:(ncx + 1) * 512],
                                start=(pr == 0), stop=(pr == 1))
                        if ncx == 1:
                            nc.scalar.copy(
                                osb[:, ncx * 512:(ncx + 1) * 512], ps_op)
                        else:
                            nc.vector.tensor_copy(
                                osb[:, ncx * 512:(ncx + 1) * 512], ps_op)
                        nc.sync.dma_start(
                            out=out_d.ap()[pq * 512 + qt * 128:
                                           pq * 512 + (qt + 1) * 128,
                                           ncx * 512:(ncx + 1) * 512],
                            in_=osb[:, ncx * 512:(ncx + 1) * 512])

    nc.compile()
    _CACHE["nc"] = nc
    return nc


def make_in_maps(x, Wq, bq, Wk, bk, Wv, bv, Wo):
    import ml_dtypes
    bf = ml_dtypes.bfloat16

    x = np.asarray(x, dtype=np.float32)
    Wq = np.asarray(Wq, dtype=np.float32)
    bq = np.asarray(bq, dtype=np.float32)
    Wk = np.asarray(Wk, dtype=np.float32)
    Wv = np.asarray(Wv, dtype=np.float32)
    bv = np.asarray(bv, dtype=np.float32)
    Wo = np.asarray(Wo, dtype=np.float32)

    # 1/8 softmax scale plus 1/4 pre-scale for the exp(4x) activation
    scale = np.float32(1.0 / 32.0)

    in_maps = []
    for core in range(N_CORES):
        b = core // 4
        g = core % 4
        cs = slice(g * G, (g + 1) * G)

        def wtile(W):  # [H, G] -> [128, 8, G] (partition-contiguous)
            return np.ascontiguousarray(
                W.reshape(8, 128, G).transpose(1, 0, 2))
        # wo: [G, H] -> [two*64+p, pr, H] stacked head pairs
        wo = Wo[cs, :].reshape(2, 2, 64, H).transpose(1, 2, 0, 3)
        in_maps.append({
            "xt": np.ascontiguousarray(
                x[b].reshape(4, 512, 8, 128).transpose(0, 3, 2, 1)).astype(bf),
            "wq": wtile(Wq[:, cs] * scale).astype(bf),
            "wk": wtile(Wk[:, cs]).astype(bf),
            "wv": wtile(Wv[:, cs]).astype(bf),
            "bq": np.ascontiguousarray(
                (bq[cs] * scale).reshape(2, 128).T.reshape(128, 2, 1)),
            "bv": np.ascontiguousarray(bv[cs].reshape(1, G)),
            "wo": np.ascontiguousarray(wo.reshape(128, 2, H)).astype(bf),
        })
    return in_maps


def kernel(x, Wq, bq, Wk, bk, Wv, bv, Wo, bo):
    from concourse.bass_utils import run_bass_kernel_spmd

    bo = np.asarray(bo, dtype=np.float32)
    nc = _build()
    in_maps = make_in_maps(x, Wq, bq, Wk, bk, Wv, bv, Wo)
    res = run_bass_kernel_spmd(nc, in_maps, core_ids=list(range(N_CORES)))

    out = np.empty((2, S, H), dtype=np.float32)
    for b in range(2):
        acc = res.results[4 * b]["out"].astype(np.float32)
        for g in range(1, 4):
            acc = acc + res.results[4 * b + g]["out"]
        out[b] = acc + bo
    return out
